# revision 2
# baseline (speedup 1.0000x reference)
"""Trainium2 Bass kernel for a binarized (1w1a) BasicBlock:

    out = relu(bn2(conv2(sign(pad(relu(bn1(conv1(sign(pad(x)), sign(w1)))))), sign(w2))) + x)

with 2x3 convs, C=256, B=64, H=W=32, pad = (W: 1 left/right, H: 1 bottom).

Strategy: data-parallel over batch across 8 NeuronCores (8 images/core).
Per core each conv is an implicit GEMM: input channels on the 128 SBUF
partitions, contraction over all 256 channels in a single PE pass via fp8e4
DoubleRow matmuls (binarized values +-1/0 are exact in fp8; PSUM accumulates
fp32, so all conv sums are exact integers). Activations live in a
"shared-pad" plane layout: 33-wide rows where one zero column serves as both
the right pad of row h and the left pad of row h+1.

v2 changes over the first HW-roofline version (105.3us):
 - the host no longer sends a separate sign(x) plane (2.39MB/core): the fp8
   residual stream x is binarized ON-CHIP into the conv1 input planes
   (ACT-engine Sign for b1..b7, a 2-op DVE chain for b0 whose ACT queue is
   busy issuing triggers).  x is "zero-bumped" on host (fp8 values that
   rounded to 0 from nonzero f32 are replaced with +-2^-9) so on-chip sign
   == host sign exactly.  This cuts total DMA from 9.5MB to 7.1MB per core:
   the two HW DMA queues were 100% busy for the whole 93us stream, so the
   freed bandwidth directly shortens the head and tail.
 - conv1/conv2 are interleaved per image (c1b0, c1b1, c2b0, c1b2, c2b1, ...)
   so output DMA starts at ~20us instead of ~53us and the tail only has to
   drain the last image.
 - every (group, 16-row chunk) gets its OWN PSUM tile: PSUM accumulation
   epochs are tracked per tile, so per-chunk tiles let each chunk's epilogue
   fire as soon as its 6 taps are done instead of waiting for the whole
   group (the old kernel lost ~2us at the tail to this).
 - the last group (b7, mt0 - conv2 runs mt1 first) is split 16/8/8 rows so
   the post-last-matmul work is one tiny epilogue + two 32KB DMA pieces.
 - b0's conv1 runs as 4x 8-row quarter-chunks fed by split DMA pieces
   (rows 0-16 / 17-31 per channel-tile) so the PE starts ~1.5us earlier;
   dependency tracking is exact per region, so quarter matmuls only wait on
   the pieces they actually read.
"""

import numpy as np
import ml_dtypes

import concourse.mybir as mybir
import concourse.tile as tile
from concourse import bacc
from concourse.bass_utils import run_bass_kernel_spmd

N_CORES = 8
B, C, H, W = 64, 256, 32, 32
BL = B // N_CORES          # images per core
P = 128
KT = C // P                # channel tiles (contraction / output)
NPOS = 6                   # 2x3 kernel taps
EPS = 1e-5
HWN = H * W

F32 = mybir.dt.float32
BF16 = mybir.dt.bfloat16
FP8 = mybir.dt.float8e4

PITCH = 33
DATA0 = 1                   # leading zero (left pad of row 0)
PLANE = DATA0 + PITCH * PITCH
NPAD = 1168                 # >= PLANE + max tap offset, mult of 16 (DoubleRow ko-stride)
R1 = 17                     # input-piece split row: piece1 = rows [0,17), piece2 = [17,32)
E1 = R1 * W                 # 544

DR = mybir.MatmulPerfMode.DoubleRow
Sign = mybir.ActivationFunctionType.Sign
Relu = mybir.ActivationFunctionType.Relu
MULT = mybir.AluOpType.mult
ADD = mybir.AluOpType.add
IS_GT = mybir.AluOpType.is_gt

_CACHE = {}


def _build():
    if "nc" in _CACHE:
        return _CACHE["nc"]

    nc = bacc.Bacc("TRN2", target_bir_lowering=False, debug=False)

    x_d = nc.dram_tensor("x", [P, BL, KT, HWN], FP8, kind="ExternalInput")
    # weights laid out [mt, ci, pos, ko, co] so per-tap slices are contiguous
    w1_d = nc.dram_tensor("w1t", [KT, P, NPOS, KT, P], FP8, kind="ExternalInput")
    w2_d = nc.dram_tensor("w2t", [KT, P, NPOS, KT, P], FP8, kind="ExternalInput")
    bnv_d = nc.dram_tensor("bnv", [4, C], F32, kind="ExternalInput")
    out_d = nc.dram_tensor("out", [BL, C, H, W], BF16, kind="ExternalOutput")

    with tile.TileContext(nc) as tc:
        with (
            tc.tile_pool(name="res", bufs=1) as res,
            tc.tile_pool(name="tmp", bufs=8) as tmp,
            tc.tile_pool(name="stg", bufs=8) as stg,
            tc.tile_pool(name="ps5", bufs=7, space="PSUM") as ps5,
        ):
            # ---- static SBUF tiles ----
            w1sb = [res.tile([P, NPOS, KT, P], FP8, tag=f"w1q{mt}", name=f"w1q{mt}")
                    for mt in range(KT)]
            w2sb = [res.tile([P, NPOS, KT, P], FP8, tag=f"w2q{mt}", name=f"w2q{mt}")
                    for mt in range(KT)]
            xg = [res.tile([P, KT, HWN], FP8, tag=f"xg{b}", name=f"xg{b}")
                  for b in range(BL)]
            xq1 = [res.tile([P, KT, NPAD], FP8, tag=f"xq1_{b}", name=f"xq1_{b}")
                   for b in range(BL)]
            xq2 = [res.tile([P, KT, NPAD], FP8, tag=f"xq2_{b}", name=f"xq2_{b}")
                   for b in range(BL)]
            bnsb = res.tile([P, 4 * KT], F32, tag="bnv", name="bnv")

            inv1sb = bnsb[:, 0 * KT:1 * KT]
            nb1sb = bnsb[:, 1 * KT:2 * KT]
            inv2sb = bnsb[:, 2 * KT:3 * KT]
            b2sb = bnsb[:, 3 * KT:4 * KT]

            def pad_memsets(q, eng):
                """Zero the pad cells of a plane tile: leading zero, shared
                pad column, bottom pad row, tail."""
                v = q[:, :, DATA0:DATA0 + PITCH * PITCH].rearrange(
                    "c k (h w) -> c k h w", w=PITCH)
                eng.memset(q[:, :, 0:DATA0], 0.0)
                eng.memset(v[:, :, :, W:PITCH], 0.0)
                eng.memset(v[:, :, H:PITCH, 0:W], 0.0)
                eng.memset(q[:, :, PLANE:NPAD], 0.0)

            def plane_rows(q, kt, r0, r):
                """Interior rows [r0, r0+r) of plane q, channel-tile kt, as
                [c, r, W]."""
                return q[:, kt, DATA0 + r0 * PITCH:DATA0 + (r0 + r) * PITCH] \
                    .rearrange("c (h w) -> c h w", w=PITCH)[:, :, 0:W]

            # ---- warmup: ~1.8us of garbage matmuls so the HAM clock-gate
            # reaches 8/8 around the time the real stream is warm ----
            wu = res.tile([P, 256], FP8, tag="wu", name="wu")
            nc.gpsimd.memset(wu[:], 0.0)
            wut = ps5.tile([P, 512], F32, tag="ps5", name="wut")
            for _ in range(16):
                nc.tensor.matmul(wut[:, 0:128], wu[:, 0:P], wu[:, 0:128],
                                 start=True, stop=True)

            # ---- pad memsets: b0/b1 planes on vector (early), rest gpsimd
            pad_memsets(xq1[0], nc.vector)
            pad_memsets(xq2[0], nc.vector)

            # ---- head DMA schedule ----
            # gpsimd SWDGE: w1/mt0 tap0 (needed first), then w2/mt1 taps
            # (tap-granular: conv2 b0/mt1 needs them from ~20us), then pads.
            xd = x_d.ap()
            nc.gpsimd.dma_start(w1sb[0][:, 0:1], w1_d.ap()[0][:, 0:1])
            for pos in range(NPOS):
                nc.gpsimd.dma_start(w2sb[1][:, pos:pos + 1],
                                    w2_d.ap()[1][:, pos:pos + 1])
            for b in range(1, BL):
                pad_memsets(xq1[b], nc.gpsimd)
                pad_memsets(xq2[b], nc.gpsimd)

            # sync HW queue: x pieces (kt0) + w1 taps 1,2 + w1/mt1 first half
            nc.sync.dma_start(xg[0][:, 0:1, 0:E1], xd[:, 0, 0:1, 0:E1])
            nc.sync.dma_start(bnsb[:], bnv_d.ap().rearrange(
                "v (t p) -> p (v t)", p=P))
            nc.sync.dma_start(w1sb[0][:, 1:2], w1_d.ap()[0][:, 1:2])
            nc.sync.dma_start(w1sb[0][:, 2:3], w1_d.ap()[0][:, 2:3])
            nc.sync.dma_start(xg[0][:, 0:1, E1:HWN], xd[:, 0, 0:1, E1:HWN])
            nc.sync.dma_start(w1sb[1][:, 0:3], w1_d.ap()[1][:, 0:3])
            nc.sync.dma_start(xg[1][:, 0:1, 0:E1], xd[:, 1, 0:1, 0:E1])
            nc.sync.dma_start(xg[1][:, 0:1, E1:HWN], xd[:, 1, 0:1, E1:HWN])

            # scalar HW queue: x pieces (kt1) + w1 taps 3-5 halves; the
            # scalar ENGINE also runs the Sign binarize for kt1 of b0/b1 in
            # between its triggers (vector handles kt0 - see below).
            nc.scalar.dma_start(xg[0][:, 1:2, 0:E1], xd[:, 0, 1:2, 0:E1])

            def bin_sign(b, kt, r0, r):
                """ACT-engine binarize: plane rows <- Sign(x rows)."""
                nc.scalar.activation(
                    plane_rows(xq1[b], kt, r0, r),
                    xg[b][:, kt, r0 * W:(r0 + r) * W].rearrange(
                        "c (h w) -> c h w", w=W),
                    Sign)

            def bin_sign_full(b):
                """Whole-image ACT binarize (both channel tiles, 4D APs)."""
                nc.scalar.activation(
                    xq1[b][:, :, DATA0:DATA0 + H * PITCH].rearrange(
                        "c k (h w) -> c k h w", w=PITCH)[:, :, :, 0:W],
                    xg[b].rearrange("c k (h w) -> c k h w", w=W),
                    Sign)

            def bin_dve(b, kt, r0, r):
                """DVE 2-op binarize for the head (ACT queue busy there):
                z = (x > 0) in {0,1}; plane = z*2 - 1 in {-1,+1}."""
                n = r * W
                bz = tmp.tile([P, E1], FP8, tag="bz", name=f"bz_{b}_{kt}_{r0}")
                nc.vector.tensor_scalar(
                    bz[:, 0:n], xg[b][:, kt, r0 * W:(r0 + r) * W],
                    0.0, None, IS_GT)
                nc.vector.tensor_scalar(
                    plane_rows(xq1[b], kt, r0, r),
                    bz[:, 0:n].rearrange("c (h w) -> c h w", w=W),
                    2.0, -1.0, MULT, ADD)

            bin_sign(0, 1, 0, R1)
            nc.scalar.dma_start(w1sb[0][:, 3:6], w1_d.ap()[0][:, 3:6])
            nc.scalar.dma_start(xg[0][:, 1:2, E1:HWN], xd[:, 0, 1:2, E1:HWN])
            bin_sign(0, 1, R1, H - R1)
            nc.scalar.dma_start(w1sb[1][:, 3:6], w1_d.ap()[1][:, 3:6])
            nc.scalar.dma_start(xg[1][:, 1:2, 0:E1], xd[:, 1, 1:2, 0:E1])
            nc.scalar.dma_start(xg[1][:, 1:2, E1:HWN], xd[:, 1, 1:2, E1:HWN])
            bin_sign(1, 1, 0, R1)
            bin_sign(1, 1, R1, H - R1)

            # ---- conv building blocks ----
            def mm_group(b, mt, wsb, src, chunks, tiles, posouter=False):
                """One output-channel-tile group: 6-tap DoubleRow
                accumulation per chunk, each chunk into its own PSUM tile.
                chunks: list of (row0, nrows)."""
                order = ([(i, pos) for pos in range(NPOS) for i in range(len(chunks))]
                         if posouter else
                         [(i, pos) for i in range(len(chunks)) for pos in range(NPOS)])
                for i, pos in order:
                    r0, r = chunks[i]
                    kh, kw = divmod(pos, 3)
                    off = kh * PITCH + kw + r0 * PITCH
                    nc.tensor.matmul(
                        tiles[i][:, 0:r * W],
                        wsb[mt][:, pos],
                        src[:, :, off: off + r * PITCH].rearrange(
                            "c k (r x) -> c k r x", x=PITCH)[:, :, :, 0:W],
                        start=(pos == 0),
                        stop=(pos == NPOS - 1),
                        perf_mode=DR,
                    )

            def c1_epi(b, mt, r0, r, pt):
                """conv1 epilogue: bn1 + relu + binarize collapse into one
                DVE tensor_scalar ((psum*inv1) is_gt -bias1 -> {0,1} fp8)
                written straight into conv2's plane rows."""
                nc.vector.tensor_scalar(
                    plane_rows(xq2[b], mt, r0, r),
                    pt[:, 0:r * W].rearrange("c (r w) -> c r w", w=W),
                    inv1sb[:, mt:mt + 1],
                    nb1sb[:, mt:mt + 1],
                    MULT,
                    IS_GT,
                )

            def c2_epi(b, mt, ot, dst, pt, row0, r0, r1, qs):
                """conv2 epilogue piece for chunk rows [r0, r1): DVE
                (psum*inv2 + x), ACT Relu(+bias2) -> bf16, DMA out on qs."""
                n0, n = (row0 + r0) * W, (r1 - r0) * W
                tt = tmp.tile([P, 512], F32, tag="t2",
                              name=f"t2_{b}_{mt}_{row0}_{r0}")
                nc.vector.scalar_tensor_tensor(
                    tt[:, 0:n],
                    pt[:, r0 * W:r1 * W],
                    inv2sb[:, mt:mt + 1],
                    xg[b][:, mt, n0:n0 + n],
                    MULT,
                    ADD,
                )
                nc.scalar.activation(
                    ot[:, n0:n0 + n], tt[:, 0:n], Relu,
                    bias=b2sb[:, mt:mt + 1], scale=1.0,
                )
                h = n // len(qs)
                for qi, q in enumerate(qs):
                    q.dma_start(dst[:, n0 + qi * h:n0 + qi * h + h],
                                ot[:, n0 + qi * h:n0 + qi * h + h])

            def conv2_group(b, mt, chunks, piece_qs):
                """conv2 group with per-chunk epilogue pieces.
                piece_qs: list per chunk of [(r0, r1, qs), ...]."""
                tiles = [ps5.tile([P, 512], F32, tag="ps5",
                                  name=f"p2_{b}_{mt}_{i}")
                         for i in range(len(chunks))]
                ot = stg.tile([P, HWN], BF16, tag="ot", name=f"ot_{b}_{mt}")
                dst = out_d.ap()[b, mt * P:(mt + 1) * P].rearrange(
                    "c h w -> c (h w)")
                for i, (r0c, r) in enumerate(chunks):
                    mm_group(b, mt, w2sb, xq2[b], [chunks[i]], [tiles[i]])
                    for (r0, r1, qs) in piece_qs[i]:
                        c2_epi(b, mt, ot, dst, tiles[i], r0c, r0, r1, qs)

            QJ16 = [(0, 16), (16, 16)]
            S, X, G = nc.sync, nc.scalar, nc.gpsimd

            # ================= block sequence =================
            # c1b0 mt0: 4x 8-row quarters, chunk-outer; kt0 binarize on DVE
            # interleaved with the quarter epilogues.
            bin_dve(0, 0, 0, R1)
            q4 = [(0, 8), (8, 8), (16, 8), (24, 8)]
            t10 = [ps5.tile([P, 512], F32, tag="ps5", name=f"p1_0_0_{i}")
                   for i in range(4)]
            mm_group(0, 0, w1sb, xq1[0], q4, t10)
            c1_epi(0, 0, 0, 8, t10[0])
            bin_dve(0, 0, R1, H - R1)
            c1_epi(0, 0, 8, 8, t10[1])
            c1_epi(0, 0, 16, 8, t10[2])
            c1_epi(0, 0, 24, 8, t10[3])

            # c1b0 mt1: quarters pos-outer (weights stream in per-tap)
            t11 = [ps5.tile([P, 512], F32, tag="ps5", name=f"p1_0_1_{i}")
                   for i in range(4)]
            mm_group(0, 1, w1sb, xq1[0], q4, t11, posouter=True)
            bin_dve(1, 0, 0, R1)
            bin_dve(1, 0, R1, H - R1)
            c1_epi(0, 1, 0, 8, t11[0])
            c1_epi(0, 1, 8, 8, t11[1])
            c1_epi(0, 1, 16, 8, t11[2])
            c1_epi(0, 1, 24, 8, t11[3])

            def conv1_group(b):
                for mt in range(KT):
                    tl = [ps5.tile([P, 512], F32, tag="ps5",
                                   name=f"p1_{b}_{mt}_{i}") for i in range(2)]
                    mm_group(b, mt, w1sb, xq1[b], QJ16, tl)
                    c1_epi(b, mt, 0, 16, tl[0])
                    c1_epi(b, mt, 16, 16, tl[1])

            def conv2_mid(b):
                for mt in (1, 0):
                    if mt == 1 and b in (0, 2, 4):
                        # SWDGE takes the first chunk whole (it lingers at
                        # ~16GB/s, so only early/mid-kernel chunks)
                        pq = [[(0, 16, (G,))],
                              [(0, 8, (X,)), (8, 16, (S,))]]
                    else:
                        a, bq = (S, X) if (b + mt) % 2 == 0 else (X, S)
                        pq = [[(0, 8, (a,)), (8, 16, (bq,))],
                              [(0, 8, (bq,)), (8, 16, (a,))]]
                    conv2_group(b, mt, QJ16, pq)

            # c1b1
            conv1_group(1)
            # late input triggers + whole-image binarize for b2
            S.dma_start(w2sb[0][:, 0:3], w2_d.ap()[0][:, 0:3])
            S.dma_start(xg[2][:], xd[:, 2])
            X.dma_start(w2sb[0][:, 3:6], w2_d.ap()[0][:, 3:6])
            X.dma_start(xg[3][:], xd[:, 3])
            bin_sign_full(2)

            # pair pipeline
            conv2_mid(0)
            S.dma_start(xg[4][:], xd[:, 4])
            X.dma_start(xg[5][:], xd[:, 5])
            conv1_group(2)
            bin_sign_full(3)
            conv2_mid(1)
            S.dma_start(xg[6][:], xd[:, 6])
            X.dma_start(xg[7][:], xd[:, 7])
            conv1_group(3)
            bin_sign_full(4)
            conv2_mid(2)
            conv1_group(4)
            bin_sign_full(5)
            conv2_mid(3)
            conv1_group(5)
            bin_sign_full(6)
            conv2_mid(4)
            conv1_group(6)
            bin_sign_full(7)
            conv2_mid(5)
            conv1_group(7)
            conv2_mid(6)

            # ---- tail: b7 conv2. mt1 first (regular chunks), then mt0 with
            # 16/8/8 rows so only a tiny epilogue trails the last matmul.
            conv2_group(7, 1, QJ16,
                        [[(0, 8, (S,)), (8, 16, (X,))],
                         [(0, 8, (X,)), (8, 16, (S,))]])
            conv2_group(7, 0, [(0, 16), (16, 8), (24, 8)],
                        [[(0, 8, (S,)), (8, 16, (X,))],
                         [(0, 8, (S,))],
                         [(0, 4, (X,)), (4, 8, (S,))]])

    nc.compile()
    _CACHE["nc"] = nc
    return nc


def _prep(w1, w2, gamma1, beta1, mean1, var1, gamma2, beta2, mean2, var2):
    """Host-side: fold BN, binarize + lay out weights as lhsT tiles."""
    def fold(gamma, beta, mean, var):
        inv = (gamma.astype(np.float64) / np.sqrt(var.astype(np.float64) + EPS))
        inv = inv.astype(np.float32)
        bias = (beta.astype(np.float32) - mean.astype(np.float32) * inv)
        return inv, bias

    inv1, bias1 = fold(gamma1, beta1, mean1, var1)
    inv2, bias2 = fold(gamma2, beta2, mean2, var2)

    def wt(w):
        # [O, I, 2, 3] -> lhsT layout [mt, ci, pos, ko, co]
        s = np.sign(w).astype(np.float32)
        arr = s.transpose(1, 2, 3, 0).reshape(KT, P, NPOS, KT, P)  # [ko,ci,pos,mt,co]
        arr = arr.transpose(3, 1, 2, 0, 4)
        return np.ascontiguousarray(arr).astype(mybir.dt.np(FP8))

    bnv = np.ascontiguousarray(np.stack([inv1, -bias1, inv2, bias2]))
    return wt(w1), wt(w2), bnv


def _in_maps(x, w1t, w2t, bnv):
    """Per-core input dicts. x streams as fp8 [p, b, kt, hw] and is both the
    residual and the on-chip binarize source; fp8 values that rounded to 0
    from nonzero f32 are bumped to +-2^-9 so Sign(x_fp8) == sign(x_f32)."""
    fp8t = mybir.dt.np(FP8)
    bump = np.float32(2.0 ** -9)
    maps = []
    for c in range(N_CORES):
        xs = x[c * BL:(c + 1) * BL]                       # [BL, C, H, W]
        xh = np.ascontiguousarray(
            xs.reshape(BL, KT, P, HWN).transpose(2, 0, 1, 3))  # [P,BL,KT,HW] f32
        xq = xh.astype(fp8t)
        zl = (xq.astype(np.float32) == 0.0) & (xh != 0.0)
        if zl.any():
            fix = np.where(xh > 0, bump, -bump).astype(fp8t)
            xq = np.where(zl, fix, xq)
        maps.append({"x": xq, "w1t": w1t, "w2t": w2t, "bnv": bnv})
    return maps


def kernel(x, w1, gamma1, beta1, mean1, var1,
           w2, gamma2, beta2, mean2, var2):
    x = np.asarray(x, dtype=np.float32)
    w1t, w2t, bnv = _prep(
        np.asarray(w1), np.asarray(w2),
        np.asarray(gamma1), np.asarray(beta1), np.asarray(mean1), np.asarray(var1),
        np.asarray(gamma2), np.asarray(beta2), np.asarray(mean2), np.asarray(var2),
    )

    nc = _build()
    in_maps = _in_maps(x, w1t, w2t, bnv)

    res = run_bass_kernel_spmd(nc, in_maps, core_ids=list(range(N_CORES)))
    out = np.concatenate([r["out"] for r in res.results], axis=0)
    return out.astype(np.float32)


# revision 4
# speedup vs baseline: 1.0560x; 1.0560x over previous
"""Trainium2 Bass kernel for a binarized (1w1a) BasicBlock:

    out = relu(bn2(conv2(sign(pad(relu(bn1(conv1(sign(pad(x)), sign(w1)))))), sign(w2))) + x)

with 2x3 convs, C=256, B=64, H=W=32, pad = (W: 1 left/right, H: 1 bottom).

Strategy: data-parallel over batch across 8 NeuronCores (8 images/core).
Per core each conv is an implicit GEMM: input channels on the 128 SBUF
partitions, contraction over all 256 channels in a single PE pass via fp8e4
DoubleRow matmuls (binarized +-1/0 exact in fp8; PSUM accumulates fp32 so
conv sums are exact). Activations live in a "shared-pad" plane layout:
33-wide rows where one zero column is both the right pad of row h and the
left pad of row h+1.

v3 structure (the two HW DMA queues were 100% busy for the entire stream in
the v1 kernel - DMA bytes, not PE work, set the head and tail):
 - images b2..b7 are binarized ON-CHIP from the fp8 residual stream x
   (kt0 via a 2-op DVE chain, kt1 via ACT-engine Sign, in parallel), saving
   6 x 299KB of host sign-plane DMA.  x is "zero-bumped" on host (fp8
   values that rounded to 0 from nonzero f32 become +-2^-9) so on-chip
   sign == host sign exactly.
 - b0/b1 still get host-packed sign planes: the on-chip binarize latency
   chain (DMA -> DVE/ACT -> PE) would push the first matmuls later than
   the plain plane DMA does; for the first two images latency wins over
   bytes, after that bytes win.
 - conv1/conv2 interleave per image (c1b0, c1b1, c2b0, c1b2, c2b1, ...) so
   output DMA starts at ~21us instead of ~53us.
 - every (group, chunk) has its OWN PSUM tile: accumulation epochs are
   tracked per tile, so per-chunk tiles let each chunk's epilogue start as
   soon as its 6 taps finish (the v1 kernel lost ~2us at the tail waiting
   for whole-group completion).
 - the final group (b7/mt0; conv2 runs mt1 first) is split 16/8/8 rows so
   only one tiny epilogue + two 32KB DMA pieces trail the last matmul; five
   mid-kernel output chunks ride the gpsimd SWDGE queue to keep the HW
   queues clear at the tail.
"""

import numpy as np
import ml_dtypes

import concourse.mybir as mybir
import concourse.tile as tile
from concourse import bacc
from concourse.bass_utils import run_bass_kernel_spmd

N_CORES = 8
B, C, H, W = 64, 256, 32, 32
BL = B // N_CORES          # images per core
P = 128
KT = C // P                # channel tiles (contraction / output)
NPOS = 6                   # 2x3 kernel taps
EPS = 1e-5
HWN = H * W
NHOST = 2                  # images with host-packed sign planes

F32 = mybir.dt.float32
BF16 = mybir.dt.bfloat16
FP8 = mybir.dt.float8e4

PITCH = 33
DATA0 = 1                   # leading zero (left pad of row 0)
PLANE = DATA0 + PITCH * PITCH
NPAD = 1168                 # >= PLANE + max tap offset, mult of 16 (DoubleRow ko-stride)

DR = mybir.MatmulPerfMode.DoubleRow
Sign = mybir.ActivationFunctionType.Sign
Relu = mybir.ActivationFunctionType.Relu
MULT = mybir.AluOpType.mult
ADD = mybir.AluOpType.add
IS_GT = mybir.AluOpType.is_gt

_CACHE = {}


def _build():
    if "nc" in _CACHE:
        return _CACHE["nc"]

    nc = bacc.Bacc("TRN2", target_bir_lowering=False, debug=False)

    x_d = nc.dram_tensor("x", [P, BL, KT, HWN], FP8, kind="ExternalInput")
    xq01_d = nc.dram_tensor("xq01", [P, NHOST, KT, NPAD], FP8, kind="ExternalInput")
    # weights laid out [mt, ci, pos, ko, co] so per-tap slices are contiguous
    w1_d = nc.dram_tensor("w1t", [KT, P, NPOS, KT, P], FP8, kind="ExternalInput")
    w2_d = nc.dram_tensor("w2t", [KT, P, NPOS, KT, P], FP8, kind="ExternalInput")
    bnv_d = nc.dram_tensor("bnv", [4, C], F32, kind="ExternalInput")
    out_d = nc.dram_tensor("out", [BL, C, H, W], BF16, kind="ExternalOutput")

    with tile.TileContext(nc) as tc:
        with (
            tc.tile_pool(name="res", bufs=1) as res,
            tc.tile_pool(name="tmp", bufs=8) as tmp,
            tc.tile_pool(name="stg", bufs=8) as stg,
            tc.tile_pool(name="ps5", bufs=7, space="PSUM") as ps5,
        ):
            # ---- static SBUF tiles ----
            w1sb = [res.tile([P, NPOS, KT, P], FP8, tag=f"w1q{mt}", name=f"w1q{mt}")
                    for mt in range(KT)]
            w2sb = [res.tile([P, NPOS, KT, P], FP8, tag=f"w2q{mt}", name=f"w2q{mt}")
                    for mt in range(KT)]
            xg = [res.tile([P, KT, HWN], FP8, tag=f"xg{b}", name=f"xg{b}")
                  for b in range(BL)]
            xq1 = [res.tile([P, KT, NPAD], FP8, tag=f"xq1_{b}", name=f"xq1_{b}")
                   for b in range(BL)]
            xq2 = [res.tile([P, KT, NPAD], FP8, tag=f"xq2_{b}", name=f"xq2_{b}")
                   for b in range(BL)]
            bnsb = res.tile([P, 4 * KT], F32, tag="bnv", name="bnv")

            inv1sb = bnsb[:, 0 * KT:1 * KT]
            nb1sb = bnsb[:, 1 * KT:2 * KT]
            inv2sb = bnsb[:, 2 * KT:3 * KT]
            b2sb = bnsb[:, 3 * KT:4 * KT]

            def pad_memsets(q, eng):
                """Zero the pad cells of a plane tile: leading zero, shared
                pad column, bottom pad row, tail."""
                v = q[:, :, DATA0:DATA0 + PITCH * PITCH].rearrange(
                    "c k (h w) -> c k h w", w=PITCH)
                eng.memset(q[:, :, 0:DATA0], 0.0)
                eng.memset(v[:, :, :, W:PITCH], 0.0)
                eng.memset(v[:, :, H:PITCH, 0:W], 0.0)
                eng.memset(q[:, :, PLANE:NPAD], 0.0)

            def plane_int(q, kt):
                """Interior of plane q, channel-tile kt, as [c, H, W]."""
                return q[:, kt, DATA0:DATA0 + H * PITCH].rearrange(
                    "c (h w) -> c h w", w=PITCH)[:, :, 0:W]

            # ---- warmup: ~3.2us of garbage matmuls so the HAM clock-gate
            # hits 8/8 right as image 0's plane lands (~10.4us) ----
            wu = res.tile([P, 256], FP8, tag="wu", name="wu")
            nc.gpsimd.memset(wu[:], 0.0)
            wut = ps5.tile([P, 512], F32, tag="ps5", name="wut")
            for _ in range(15):
                nc.tensor.matmul(wut[:, 0:256], wu[:, 0:P], wu[:],
                                 start=True, stop=True)

            pad_memsets(xq2[0], nc.vector)

            # ---- DMA schedule ----
            # gpsimd SWDGE: w1/mt0 tap0 (needed first), all of w2/mt1
            # (needed from ~21us), then the pad memsets for b1..b7.
            xd = x_d.ap()
            nc.gpsimd.dma_start(w1sb[0][:, 0:1], w1_d.ap()[0][:, 0:1])
            nc.gpsimd.dma_start(w2sb[1][:], w2_d.ap()[1])
            pad_memsets(xq2[1], nc.gpsimd)
            for b in range(NHOST, BL):
                pad_memsets(xq1[b], nc.gpsimd)
                pad_memsets(xq2[b], nc.gpsimd)

            # sync HW queue: b0/b1 plane ko0-halves + w1 taps + w2/mt0
            # first half + the kt0 halves of the x stream.
            nc.sync.dma_start(xq1[0][:, 0:1], xq01_d.ap()[:, 0, 0:1])
            nc.sync.dma_start(bnsb[:], bnv_d.ap().rearrange(
                "v (t p) -> p (v t)", p=P))
            nc.sync.dma_start(w1sb[0][:, 1:2], w1_d.ap()[0][:, 1:2])
            nc.sync.dma_start(w1sb[0][:, 2:3], w1_d.ap()[0][:, 2:3])
            nc.sync.dma_start(w1sb[1][:, 0:1], w1_d.ap()[1][:, 0:1])
            nc.sync.dma_start(w1sb[1][:, 2:3], w1_d.ap()[1][:, 2:3])
            nc.sync.dma_start(w1sb[1][:, 4:5], w1_d.ap()[1][:, 4:5])
            nc.sync.dma_start(xq1[1][:, 0:1], xq01_d.ap()[:, 1, 0:1])
            nc.sync.dma_start(w2sb[0][:, 0:3], w2_d.ap()[0][:, 0:3])
            nc.sync.dma_start(xg[0][:, 0:1], xd[:, 0, 0:1])
            nc.sync.dma_start(xg[2][:, 0:1], xd[:, 2, 0:1])
            nc.sync.dma_start(xg[1][:, 0:1], xd[:, 1, 0:1])
            nc.sync.dma_start(xg[3][:, 0:1], xd[:, 3, 0:1])
            nc.sync.dma_start(xg[4][:, 0:1], xd[:, 4, 0:1])
            nc.sync.dma_start(xg[5][:, 0:1], xd[:, 5, 0:1])
            nc.sync.dma_start(xg[6][:, 0:1], xd[:, 6, 0:1])
            nc.sync.dma_start(xg[7][:, 0:1], xd[:, 7, 0:1])

            # scalar HW queue: the ko1/kt1 halves of everything
            nc.scalar.dma_start(xq1[0][:, 1:2], xq01_d.ap()[:, 0, 1:2])
            nc.scalar.dma_start(w1sb[0][:, 3:6], w1_d.ap()[0][:, 3:6])
            nc.scalar.dma_start(w1sb[1][:, 1:2], w1_d.ap()[1][:, 1:2])
            nc.scalar.dma_start(w1sb[1][:, 3:4], w1_d.ap()[1][:, 3:4])
            nc.scalar.dma_start(w1sb[1][:, 5:6], w1_d.ap()[1][:, 5:6])
            nc.scalar.dma_start(xq1[1][:, 1:2], xq01_d.ap()[:, 1, 1:2])
            nc.scalar.dma_start(w2sb[0][:, 3:6], w2_d.ap()[0][:, 3:6])
            nc.scalar.dma_start(xg[0][:, 1:2], xd[:, 0, 1:2])
            nc.scalar.dma_start(xg[2][:, 1:2], xd[:, 2, 1:2])
            nc.scalar.dma_start(xg[1][:, 1:2], xd[:, 1, 1:2])
            nc.scalar.dma_start(xg[3][:, 1:2], xd[:, 3, 1:2])
            nc.scalar.dma_start(xg[4][:, 1:2], xd[:, 4, 1:2])
            nc.scalar.dma_start(xg[5][:, 1:2], xd[:, 5, 1:2])
            nc.scalar.dma_start(xg[6][:, 1:2], xd[:, 6, 1:2])
            nc.scalar.dma_start(xg[7][:, 1:2], xd[:, 7, 1:2])

            # ---- on-chip binarize for b2..b7: kt0 on DVE (2-op chain:
            # {0,1} then *2-1), kt1 on ACT (Sign), running in parallel ----
            def binarize(b):
                bz = tmp.tile([P, HWN], FP8, tag="bz", name=f"bz_{b}")
                nc.vector.tensor_scalar(bz[:], xg[b][:, 0], 0.0, None, IS_GT)
                nc.vector.tensor_scalar(
                    plane_int(xq1[b], 0),
                    bz[:].rearrange("c (h w) -> c h w", w=W),
                    2.0, -1.0, MULT, ADD)
                nc.scalar.activation(
                    plane_int(xq1[b], 1),
                    xg[b][:, 1].rearrange("c (h w) -> c h w", w=W),
                    Sign)

            # ---- conv building blocks ----
            def mm_group(b, mt, wsb, src, chunks, tiles, posouter=False):
                """6-tap DoubleRow accumulation per chunk, each chunk into
                its own PSUM tile. chunks: list of (row0, nrows)."""
                order = ([(i, pos) for pos in range(NPOS) for i in range(len(chunks))]
                         if posouter else
                         [(i, pos) for i in range(len(chunks)) for pos in range(NPOS)])
                for i, pos in order:
                    r0, r = chunks[i]
                    kh, kw = divmod(pos, 3)
                    off = kh * PITCH + kw + r0 * PITCH
                    nc.tensor.matmul(
                        tiles[i][:, 0:r * W],
                        wsb[mt][:, pos],
                        src[:, :, off: off + r * PITCH].rearrange(
                            "c k (r x) -> c k r x", x=PITCH)[:, :, :, 0:W],
                        start=(pos == 0),
                        stop=(pos == NPOS - 1),
                        perf_mode=DR,
                    )

            def c1_epi(b, mt, r0, r, pt):
                """conv1 epilogue: bn1 + relu + binarize collapse into one
                DVE tensor_scalar ((psum*inv1) is_gt -bias1 -> {0,1} fp8)
                written straight into conv2's plane rows."""
                nc.vector.tensor_scalar(
                    xq2[b][:, mt, DATA0 + r0 * PITCH:DATA0 + (r0 + r) * PITCH]
                    .rearrange("c (h w) -> c h w", w=PITCH)[:, :, 0:W],
                    pt[:, 0:r * W].rearrange("c (r w) -> c r w", w=W),
                    inv1sb[:, mt:mt + 1],
                    nb1sb[:, mt:mt + 1],
                    MULT,
                    IS_GT,
                )

            def conv1_group(b, posouter=False):
                for mt in range(KT):
                    tl = [ps5.tile([P, 512], F32, tag="ps5",
                                   name=f"p1_{b}_{mt}_{i}") for i in range(2)]
                    mm_group(b, mt, w1sb, xq1[b], [(0, 16), (16, 16)], tl,
                             posouter=posouter)
                    c1_epi(b, mt, 0, 16, tl[0])
                    c1_epi(b, mt, 16, 16, tl[1])

            def c2_epi(b, mt, ot, dst, pt, row0, r0, r1, qs):
                """conv2 epilogue piece for chunk rows [r0, r1): DVE
                (psum*inv2 + x), ACT Relu(+bias2) -> bf16, DMA out on qs."""
                n0, n = (row0 + r0) * W, (r1 - r0) * W
                tt = tmp.tile([P, 512], F32, tag="t2",
                              name=f"t2_{b}_{mt}_{row0}_{r0}")
                nc.vector.scalar_tensor_tensor(
                    tt[:, 0:n],
                    pt[:, r0 * W:r1 * W],
                    inv2sb[:, mt:mt + 1],
                    xg[b][:, mt, n0:n0 + n],
                    MULT,
                    ADD,
                )
                nc.scalar.activation(
                    ot[:, n0:n0 + n], tt[:, 0:n], Relu,
                    bias=b2sb[:, mt:mt + 1], scale=1.0,
                )
                h = n // len(qs)
                for qi, q in enumerate(qs):
                    q.dma_start(dst[:, n0 + qi * h:n0 + qi * h + h],
                                ot[:, n0 + qi * h:n0 + qi * h + h])

            def conv2_group(b, mt, chunks, piece_qs):
                tiles = [ps5.tile([P, 512], F32, tag="ps5",
                                  name=f"p2_{b}_{mt}_{i}")
                         for i in range(len(chunks))]
                ot = stg.tile([P, HWN], BF16, tag="ot", name=f"ot_{b}_{mt}")
                dst = out_d.ap()[b, mt * P:(mt + 1) * P].rearrange(
                    "c h w -> c (h w)")
                for i in range(len(chunks)):
                    mm_group(b, mt, w2sb, xq2[b], [chunks[i]], [tiles[i]])
                    for (r0, r1, qs) in piece_qs[i]:
                        c2_epi(b, mt, ot, dst, tiles[i], chunks[i][0], r0, r1, qs)

            QJ16 = [(0, 16), (16, 16)]
            S, X, G = nc.sync, nc.scalar, nc.gpsimd

            def conv2_mid(b):
                for mt in (1, 0):
                    if mt == 1 and b in (0, 2, 4, 5, 6):
                        # SWDGE takes the first chunk whole - it lingers at
                        # ~17GB/s but frees the HW queues for the tail
                        pq = [[(0, 16, (G,))],
                              [(0, 8, (X,)), (8, 16, (S,))]]
                    else:
                        a, bq = (S, X) if (b + mt) % 2 == 0 else (X, S)
                        pq = [[(0, 8, (a,)), (8, 16, (bq,))],
                              [(0, 8, (bq,)), (8, 16, (a,))]]
                    conv2_group(b, mt, QJ16, pq)

            # ================= block sequence =================
            conv1_group(0, posouter=True)   # taps trickle in per-slot
            binarize(2)
            conv1_group(1)
            binarize(3)
            conv2_mid(0)
            conv1_group(2)
            binarize(4)
            conv2_mid(1)
            conv1_group(3)
            binarize(5)
            conv2_mid(2)
            conv1_group(4)
            binarize(6)
            conv2_mid(3)
            conv1_group(5)
            binarize(7)
            conv2_mid(4)
            conv1_group(6)
            conv2_mid(5)
            conv1_group(7)
            conv2_mid(6)

            # ---- tail: b7 conv2. mt1 first (regular chunks), then mt0 with
            # 16/8/8 rows so only a tiny epilogue trails the last matmul.
            conv2_group(7, 1, QJ16,
                        [[(0, 8, (S,)), (8, 16, (X,))],
                         [(0, 8, (X,)), (8, 16, (S,))]])
            conv2_group(7, 0, [(0, 16), (16, 8), (24, 8)],
                        [[(0, 8, (S,)), (8, 16, (X,))],
                         [(0, 8, (S,))],
                         [(0, 4, (X,)), (4, 8, (S,))]])

    nc.compile()
    _CACHE["nc"] = nc
    return nc


def _prep(w1, w2, gamma1, beta1, mean1, var1, gamma2, beta2, mean2, var2):
    """Host-side: fold BN, binarize + lay out weights as lhsT tiles."""
    def fold(gamma, beta, mean, var):
        inv = (gamma.astype(np.float64) / np.sqrt(var.astype(np.float64) + EPS))
        inv = inv.astype(np.float32)
        bias = (beta.astype(np.float32) - mean.astype(np.float32) * inv)
        return inv, bias

    inv1, bias1 = fold(gamma1, beta1, mean1, var1)
    inv2, bias2 = fold(gamma2, beta2, mean2, var2)

    def wt(w):
        # [O, I, 2, 3] -> lhsT layout [mt, ci, pos, ko, co]
        s = np.sign(w).astype(np.float32)
        arr = s.transpose(1, 2, 3, 0).reshape(KT, P, NPOS, KT, P)  # [ko,ci,pos,mt,co]
        arr = arr.transpose(3, 1, 2, 0, 4)
        return np.ascontiguousarray(arr).astype(mybir.dt.np(FP8))

    bnv = np.ascontiguousarray(np.stack([inv1, -bias1, inv2, bias2]))
    return wt(w1), wt(w2), bnv


# flat positions of the plane interior (row h, col c) -> DATA0 + h*PITCH + c
_INT_COLS = (DATA0 + (np.arange(H)[:, None] * PITCH + np.arange(W))).ravel()


def _in_maps(x, w1t, w2t, bnv):
    """Per-core input dicts. x streams as fp8 [p, b, kt, hw]: it is both the
    residual and the binarize source for b2..b7 (fp8 values that rounded to
    0 from nonzero f32 are bumped to +-2^-9 so Sign(x_fp8) == sign(x_f32)).
    b0/b1 additionally get host-packed sign planes (latency, not bytes,
    matters for the first two images)."""
    fp8t = mybir.dt.np(FP8)
    bump = np.float32(2.0 ** -9)
    maps = []
    for c in range(N_CORES):
        xs = x[c * BL:(c + 1) * BL]                       # [BL, C, H, W]
        xh = np.ascontiguousarray(
            xs.reshape(BL, KT, P, HWN).transpose(2, 0, 1, 3))  # [P,BL,KT,HW] f32
        xq = xh.astype(fp8t)
        zl = (xq.astype(np.float32) == 0.0) & (xh != 0.0)
        if zl.any():
            fix = np.where(xh > 0, bump, -bump).astype(fp8t)
            xq = np.where(zl, fix, xq)
        plane = np.zeros((P, NHOST, KT, NPAD), np.float32)
        plane[:, :, :, _INT_COLS] = np.sign(xh[:, :NHOST])
        maps.append({"x": xq, "xq01": plane.astype(fp8t),
                     "w1t": w1t, "w2t": w2t, "bnv": bnv})
    return maps


def kernel(x, w1, gamma1, beta1, mean1, var1,
           w2, gamma2, beta2, mean2, var2):
    x = np.asarray(x, dtype=np.float32)
    w1t, w2t, bnv = _prep(
        np.asarray(w1), np.asarray(w2),
        np.asarray(gamma1), np.asarray(beta1), np.asarray(mean1), np.asarray(var1),
        np.asarray(gamma2), np.asarray(beta2), np.asarray(mean2), np.asarray(var2),
    )

    nc = _build()
    in_maps = _in_maps(x, w1t, w2t, bnv)

    res = run_bass_kernel_spmd(nc, in_maps, core_ids=list(range(N_CORES)))
    out = np.concatenate([r["out"] for r in res.results], axis=0)
    return out.astype(np.float32)
